# revision 2
# baseline (speedup 1.0000x reference)
"""Trainium2 Bass kernel for DynamicRoutingLayer.

Reference computation (the N_ITER loop is degenerate: logits do not depend on
rw, so the final rw is just softmax of the once-computed logits):
    L[b,h,n,m] = (x[b] @ W[h] @ x[b].T) * D**-0.5
    P = softmax(L, axis=-1)
    out[b]     = mean_h(P[b,h] @ x[b])

Sharding: data-parallel over B (8 batches -> 8 cores), W replicated.

Kernel per core (batch b), all matmuls in float32r (full-rate fp32,
~11-bit-mantissa input rounding):
    yT_h = (x_b @ W_h)^T        via matmul(lhsT=W_h, rhs=xT_b)   [512,1024]
    L    = yT_h^T @ xT_b        per n-tile -> PSUM [128,1024]
    softmax: DVE partial reduce_max over 128 cols (+40 safety margin;
             softmax is shift-invariant so any c within ~80 of the true
             row max is numerically safe) -> ACT Exp(bias=-c,
             accum_out=rowsum) -> DVE reciprocal
    P_sum = sum_h P_h accumulated on DVE via fused scalar_tensor_tensor
            (out = mean_h(P_h) @ x by linearity -> 4x fewer transposes
            and out-matmuls than per-head P @ x)
    per n-tile tail: 8 TensorE transposes of P_sum -> PSUM -> SBUF,
            8 out-matmuls accumulating over m into one PSUM bank.

Host-side folds: D**-0.5 into W; the 1/H head-mean into the "x" operand
(x/4) used by the out matmul.
"""

import sys

if "/opt/trn_rl_repo" not in sys.path:
    sys.path.insert(0, "/opt/trn_rl_repo")

import numpy as np

import concourse.mybir as mybir
from concourse import bacc
from concourse.bass import ts
from concourse.masks import make_identity
from concourse.tile import TileContext
from concourse.bass_utils import run_bass_kernel_spmd

B, N, D = 8, 1024, 512
H = 4
P = 128
NT = N // P       # 8 n-tiles (query rows)
MT = N // P       # 8 m-tiles (key rows)
KT = D // P       # 4 contraction tiles
NCH = N // 512    # 2 chunks of 512 along the N (m) free axis
F32 = mybir.dt.float32
F32R = mybir.dt.float32r


def build_kernel(reps=1, tail_h=1, pool_acc=False, split_tail=False):
    nc = bacc.Bacc("TRN2", target_bir_lowering=False)

    x_d = nc.dram_tensor("x", [N, D], F32R, kind="ExternalInput")
    xt_d = nc.dram_tensor("xT", [D, N], F32R, kind="ExternalInput")
    w_d = nc.dram_tensor("W", [H, D, D], F32R, kind="ExternalInput")
    o_d = nc.dram_tensor("out", [N, D], F32, kind="ExternalOutput")

    o_tiled = o_d.rearrange("(t p) d -> t p d", p=P)

    from contextlib import ExitStack

    with TileContext(nc) as tc, ExitStack() as stack:
        if reps > 1:
            stack.enter_context(
                tc.For_i(
                    0,
                    reps,
                    1,
                    hint_engines=(
                        mybir.EngineType.PE,
                        mybir.EngineType.Activation,
                        mybir.EngineType.DVE,
                        mybir.EngineType.Pool,
                        mybir.EngineType.SP,
                    ),
                )
            )
        with (
            tc.tile_pool(name="const", bufs=1) as const,
            tc.tile_pool(name="ypool", bufs=1) as ypool,
            tc.tile_pool(name="psum_big", bufs=3, space="PSUM") as psum_big,
            tc.tile_pool(name="psum_t1", bufs=1, space="PSUM") as psum_t1,
            tc.tile_pool(name="psum_o", bufs=1, space="PSUM") as psum_o,
            tc.tile_pool(name="stat", bufs=4) as stat,
            tc.tile_pool(name="epool", bufs=3) as epool,
            tc.tile_pool(name="enpool", bufs=4) as enpool,
            tc.tile_pool(name="ptpool", bufs=3) as ptpool,
            tc.tile_pool(name="outpool", bufs=3) as outpool,
        ):
            identity_f32 = const.tile([P, P], F32)
            make_identity(nc, identity_f32)
            identity = const.tile([P, P], F32R)
            nc.vector.tensor_copy(identity, identity_f32)

            # load order: xT + W feed the Y phase (first matmuls); x_nat is
            # not needed until the first pipeline tail, so it loads last.
            xt_sb = const.tile([P, KT, N], F32R)   # [p, k-tile, n]
            xt_re = xt_d.rearrange("(k p) n -> k p n", p=P)
            for k in range(KT):
                nc.sync.dma_start(out=xt_sb[:, k], in_=xt_re[k])
            # W split per head so head 0 is ready as soon as possible
            w_sb = const.tile([P, H, KT, D], F32R)  # [p, h, k-tile, e]
            w_re = w_d.rearrange("h (k p) e -> h p k e", p=P)
            for h in range(H):
                nc.sync.dma_start(out=w_sb[:, h], in_=w_re[h])
            x_nat = const.tile([P, MT, D], F32R)   # [p, m-tile, d]
            nc.sync.dma_start(
                out=x_nat, in_=x_d.rearrange("(t p) d -> p t d", p=P)
            )

            # yT[h] = (x @ W_h)^T, stored [p, h, e-tile, n]
            yt_sb = ypool.tile([P, H, KT, N], F32R)
            for h in range(H):
                for e in range(KT):
                    ps = psum_big.tile([P, N], F32, tag="big")
                    for nch in range(NCH):
                        for k in range(KT):
                            nc.tensor.matmul(
                                ps[:, ts(nch, 512)],
                                lhsT=w_sb[:, h, k, ts(e, P)],
                                rhs=xt_sb[:, k, ts(nch, 512)],
                                start=(k == 0),
                                stop=(k == KT - 1),
                            )
                        # copy per 512-chunk so each PSUM bank is released as
                        # soon as its accumulation group retires
                        nc.vector.tensor_copy(
                            yt_sb[:, h, e, ts(nch, 512)], ps[:, ts(nch, 512)]
                        )

            # main loop: per n-tile, accumulate P_sum = sum_h P_h on DVE
            # (out = mean_h(P_h) @ x by linearity), then one transpose+matmul
            # tail per n-tile.  Tails deferred ~2 (nt,h)-steps so the softmax
            # chain latency is covered by PE work.
            pending = []

            def emit_half_w1(nt, pacc, pt):
                pt_ps = psum_t1.tile([P, 512], F32R, name="pt_ps1", tag="pt_ps1")
                for q in range(4):
                    nc.tensor.transpose(
                        pt_ps[:, ts(q, P)], pacc[:, ts(q, P)], identity
                    )
                nc.vector.tensor_copy(pt[:, 0:4, :], pt_ps)

            def emit_half(nt, pacc, half, tpool, pt):
                pt_ps = tpool.tile([P, 512], F32R, name="pt_ps", tag="pt_ps1")
                for q in range(4):
                    mt = half * 4 + q
                    nc.tensor.transpose(
                        pt_ps[:, ts(q, P)], pacc[:, ts(mt, P)], identity
                    )
                nc.vector.tensor_copy(pt[:, half * 4 : half * 4 + 4, :], pt_ps)

            def emit_finish(nt, pt):
                po = psum_o.tile([P, D], F32, name="po")
                for mt in range(MT):
                    nc.tensor.matmul(
                        po,
                        lhsT=pt[:, mt, :],
                        rhs=x_nat[:, mt, :],
                        start=(mt == 0),
                        stop=(mt == MT - 1),
                    )
                osb = outpool.tile([P, D], F32)
                nc.vector.tensor_copy(osb, po)
                nc.sync.dma_start(out=o_tiled[nt], in_=osb)

            def emit_tail(nt, pacc):
                pt = ptpool.tile([P, MT, P], F32R)
                emit_half_w1(nt, pacc, pt)
                emit_half(nt, pacc, 1, psum_t1, pt)
                emit_finish(nt, pt)

            pacc = None
            for nt in range(NT):
                for h in range(H):
                    psl = psum_big.tile([P, N], F32, tag="big")
                    for mch in range(NCH):
                        for k in range(KT):
                            nc.tensor.matmul(
                                psl[:, ts(mch, 512)],
                                lhsT=yt_sb[:, h, k, ts(nt, P)],
                                rhs=xt_sb[:, k, ts(mch, 512)],
                                start=(k == 0),
                                stop=(k == KT - 1),
                            )
                    negmax = stat.tile([P, 1], F32)
                    nc.vector.reduce_max(
                        negmax, psl[:, 0:P], axis=mybir.AxisListType.X, negate=True
                    )
                    nc.vector.tensor_scalar_add(negmax, negmax, -40.0)
                    e_t = epool.tile([P, N], F32)
                    ssum = stat.tile([P, 1], F32)
                    nc.scalar.activation(
                        out=e_t,
                        in_=psl,
                        func=mybir.ActivationFunctionType.Exp,
                        bias=negmax,
                        scale=1.0,
                        accum_out=ssum,
                    )
                    rinv = stat.tile([P, 1], F32)
                    nc.vector.reciprocal(rinv, ssum)
                    if h == 0:
                        pacc = enpool.tile([P, N], F32R, name="pacc")
                        nc.vector.tensor_scalar_mul(pacc, e_t, rinv)
                    elif pool_acc:
                        # normalize on DVE, accumulate on the idle GpSimd so
                        # the DVE pipe-drain of a fused 2-input op is avoided
                        en_h = epool.tile([P, N], F32, name="en_h")
                        nc.vector.tensor_scalar_mul(en_h, e_t, rinv)
                        nc.gpsimd.tensor_add(pacc, pacc, en_h)
                    else:
                        # pacc += e_t * rinv, fused
                        nc.vector.scalar_tensor_tensor(
                            out=pacc,
                            in0=e_t,
                            scalar=rinv,
                            in1=pacc,
                            op0=mybir.AluOpType.mult,
                            op1=mybir.AluOpType.add,
                        )
                    if h == H - 1:
                        pending.append((nt, pacc))
                    if split_tail:
                        # spread the tail: half0 at h==tail_h, half1+out one
                        # step later, smoothing the DVE load per step
                        if pending and h == tail_h:
                            tnt, tpacc = pending[0]
                            tpt = ptpool.tile([P, MT, P], F32R, name="pt")
                            emit_half(tnt, tpacc, 0, psum_t1, tpt)
                            pending[0] = (tnt, tpacc, tpt)
                        elif pending and h == tail_h + 1:
                            tnt, tpacc, tpt = pending.pop(0)
                            emit_half(tnt, tpacc, 1, psum_t2, tpt)
                            emit_finish(tnt, tpt)
                    else:
                        if pending and (h == tail_h):
                            emit_tail(*pending.pop(0))
            for p in pending:
                if len(p) == 3:
                    tnt, tpacc, tpt = p
                    emit_half(tnt, tpacc, 1, psum_t2, tpt)
                    emit_finish(tnt, tpt)
                else:
                    emit_tail(*p)

    nc.compile()
    return nc


_NC_CACHE = None


def make_in_maps(x, W):
    x = np.asarray(x, dtype=np.float32)
    W = np.asarray(W, dtype=np.float32)
    scale = np.float32(D ** -0.5)
    w_scaled = np.ascontiguousarray(W * scale)

    in_maps = []
    for b in range(B):
        xb = np.ascontiguousarray(x[b])
        in_maps.append(
            {
                # 1/H head-mean folded into the out-matmul operand
                "x": np.ascontiguousarray(xb * np.float32(1.0 / H)),
                "xT": np.ascontiguousarray(xb.T),
                "W": w_scaled,
            }
        )
    return in_maps


def kernel(x, W):
    global _NC_CACHE
    if _NC_CACHE is None:
        _NC_CACHE = build_kernel()
    nc = _NC_CACHE

    in_maps = make_in_maps(x, W)
    res = run_bass_kernel_spmd(nc, in_maps, core_ids=list(range(B)))
    out = np.stack([res.results[b]["out"] for b in range(B)], axis=0)
    return out

